# revision 28
# baseline (speedup 1.0000x reference)
"""Trainium2 Bass kernel for sparse_attention (nn_Attention_77446850282057).

B=8, H=4, L=256, DK=64; 2 global heads + 2 local heads with per-head MLP
reweighting of pairwise (rel_pos + v_i + v_j) features.

Math: the MLP is linear, so
  reweight[b,n,i,j] = R[n,i,j] + a[b,n,i] + a[b,n,j] + mlp_b[n]
with R[n,i,j] = rel_pos[j-i+L-1, nDK:] . w[n]   (Toeplitz in (i,j))
and  a[b,n,j] = v_l[b,n,j] . w[n].
Row-constant terms (a[...,i] and mlp_b) shift softmax rows uniformly and
drop out exactly.  Softmax is computed without max-subtraction (scores are
bounded ~|S|<15 so raw exp is safe in f32); masked entries get -1e4 added
which underflows exp to exactly 0, matching the reference's where(mask,-1e4).

Sharding: pure data-parallel over batch, 1 batch per NeuronCore, 8 cores.

Device notes (per core):
 - host passes q/k/v/rel_pos transposed (d-major) — no input transposes
 - S tile (i-part, j-free) accumulated in PSUM by chained matmuls:
     qk (contract 64) + rank-1 a[j] (ones^T @ a2) + one J^T matmul adding
     mask penalty and Toeplitz together (host flips mask rows; J@J=I)
 - p = exp(S) via ACT with accum_out row sums; P^T via PE transposes of
   unnormalized p; O normalized by 1/rowsum in the PSUM->SBUF copy
 - DMA issue spread across sync (inputs), scalar (inputs), gpsimd
   (rel scratch round-trip + outputs) to avoid sequencer serialization
"""

import numpy as np

B, H, L, DK = 8, 4, 256, 64
GN, LN = 2, 2
SCALE = 1.0 / 8.0
NCORES = 8

_CACHE = {}


def _build_nc():
    import concourse.bass as bass
    import concourse.tile as tile
    from concourse import bacc, mybir
    from concourse.bass import ts
    from concourse.masks import make_identity

    f32 = mybir.dt.float32
    bf16 = mybir.dt.bfloat16
    u8 = mybir.dt.uint8

    # Bacc (not Bass): its compile pass splits multi-sem waits into
    # EventSemaphore instructions — TRN2 structs take at most one wait
    nc = bacc.Bacc()

    qT = nc.dram_tensor("qT", [128, 2, 256], f32, kind="ExternalInput")
    kT = nc.dram_tensor("kT", [128, 2, 256], f32, kind="ExternalInput")
    vn = nc.dram_tensor("vn", [128, 4, 2, 64], f32, kind="ExternalInput")
    vTl = nc.dram_tensor("vTl", [128, 256], f32, kind="ExternalInput")
    msk = nc.dram_tensor("msk", [128, 2, 256], u8, kind="ExternalInput")
    aqT = nc.dram_tensor("aqT", [128, 256], f32, kind="ExternalInput")
    akT = nc.dram_tensor("akT", [128, 256], f32, kind="ExternalInput")
    rpT = nc.dram_tensor("rpT", [128, 511], f32, kind="ExternalInput")
    # host-scattered weights: w2[n*64+d, n*64] = mlp_w[n, d], zeros elsewhere
    w2in = nc.dram_tensor("w2", [128, 65], f32, kind="ExternalInput")

    outp = nc.dram_tensor("out", [128, 4, 2, 64], f32, kind="ExternalOutput")
    rrd = nc.dram_tensor("rr_scratch", [2 * 511], bf16)
    pg = nc.dram_tensor("p_g", [2, 128, 2, 256], bf16, kind="ExternalOutput")
    pl = nc.dram_tensor("p_l", [2, 128, 2, 256], bf16, kind="ExternalOutput")

    Exp = mybir.ActivationFunctionType.Exp

    with tile.TileContext(nc) as tc:
        with (
            tc.tile_pool(name="const", bufs=1) as const,
            tc.tile_pool(name="io", bufs=1) as io,
            tc.tile_pool(name="work", bufs=2) as work,
            tc.tile_pool(name="ps", bufs=1, space="PSUM") as ps,
        ):
            # ---------- input loads: two HWDGE rings, small gating loads first
            mask_u = io.tile([128, 2, 256], u8)
            nc.sync.dma_start(out=mask_u, in_=msk[:])
            w2 = const.tile([128, 65], f32)
            nc.sync.dma_start(out=w2, in_=w2in[:])
            rpT_f = io.tile([128, 511], f32)
            nc.sync.dma_start(out=rpT_f, in_=rpT[:])
            v_f = io.tile([128, 4, 2, 64], f32)
            nc.sync.dma_start(out=v_f, in_=vn[:])

            qT_f = io.tile([128, 2, 256], f32)
            nc.scalar.dma_start(out=qT_f, in_=qT[:])
            kT_f = io.tile([128, 2, 256], f32)
            nc.scalar.dma_start(out=kT_f, in_=kT[:])
            aqT_f = io.tile([128, 256], f32)
            nc.scalar.dma_start(out=aqT_f, in_=aqT[:])
            akT_f = io.tile([128, 256], f32)
            nc.scalar.dma_start(out=akT_f, in_=akT[:])
            vTl_f = io.tile([128, 256], f32)
            nc.scalar.dma_start(out=vTl_f, in_=vTl[:])

            # ---------- constants
            idb = const.tile([128, 128], bf16)  # identity for PE transposes
            make_identity(nc, idb)
            Jt = const.tile([128, 128], bf16)  # anti-identity J[p,y]=1 iff p+y=127
            nc.gpsimd.memset(Jt, 0.0)
            nc.gpsimd.affine_select(
                out=Jt, in_=Jt, compare_op=mybir.AluOpType.not_equal,
                fill=1.0, base=-127, pattern=[[1, 128]], channel_multiplier=1,
            )
            onesb = const.tile([1, 128], bf16)
            nc.gpsimd.memset(onesb, 1.0)

            # ---------- DVE: q/k bf16 prep first (gates both waves' qk mms)
            qT_b = io.tile([128, 2, 256], bf16)
            kT_b = io.tile([128, 2, 256], bf16)
            nc.vector.tensor_scalar_mul(qT_b[:, 0, :], qT_f[:, 0, :], SCALE)
            nc.vector.tensor_copy(out=kT_b[:, 0, :], in_=kT_f[:, 0, :])
            qlt = io.tile([128, 256], f32)
            nc.vector.tensor_tensor(qlt, qT_f[:, 1, :], aqT_f, mybir.AluOpType.add)
            klt = io.tile([128, 256], f32)
            nc.vector.tensor_tensor(klt, kT_f[:, 1, :], akT_f, mybir.AluOpType.add)
            nc.vector.tensor_scalar_mul(qT_b[:, 1, :], qlt, SCALE)
            nc.vector.tensor_copy(out=kT_b[:, 1, :], in_=klt)

            # ---------- ACT: rel-chain casts + mask penalty (DVE queue is long)
            w2b = const.tile([128, 65], bf16)
            nc.scalar.copy(out=w2b, in_=w2)
            mneg_b = io.tile([128, 2, 256], bf16)
            nc.scalar.activation(
                out=mneg_b, in_=mask_u,
                func=mybir.ActivationFunctionType.Copy, scale=-10000.0,
            )
            rpT_b = io.tile([128, 511], bf16)
            nc.scalar.copy(out=rpT_b, in_=rpT_f)

            # ---------- DVE: remaining converts + combined local J-rhs
            vT_b = io.tile([128, 256], bf16)
            nc.vector.tensor_copy(out=vT_b, in_=vTl_f)
            v_b = io.tile([128, 4, 2, 64], bf16)
            nc.vector.tensor_copy(out=v_b, in_=v_f)

            # ---------- a2 = w . v_l (both local heads, partitions 0/64)
            a2_ps = ps.tile([65, 256], f32, tag="s", bufs=4)
            nc.tensor.matmul(a2_ps, lhsT=w2b, rhs=vT_b, start=True, stop=True)
            a2_sb = [const.tile([1, 256], bf16, name=f"a2_{n}") for n in range(2)]
            for n in range(2):
                nc.vector.tensor_copy(out=a2_sb[n], in_=a2_ps[n * 64:n * 64 + 1, :])

            # ---------- attention: all S matmuls emitted in data-readiness
            # order (PE sequencers run in-order; a stalled op blocks the queue)
            o_all = const.tile([128, 4, 2, 64], f32)
            s_ps, p_sb, p_out, z, rz, pT = {}, {}, {}, {}, {}, {}
            tiles = [(hl, ib) for hl in range(2) for ib in range(2)]

            def emit_qk(hp):
                for hl, ib in tiles:
                    sp = ps.tile(
                        [128, 256], f32, tag="s", bufs=4, name=f"s_{hp}_{hl}_{ib}"
                    )
                    s_ps[hp, hl, ib] = sp
                    sq = qT_b[hl * 64:(hl + 1) * 64, hp, :]
                    sk = kT_b[hl * 64:(hl + 1) * 64, hp, :]
                    nc.tensor.matmul(
                        sp, lhsT=sq[:, ts(ib, 128)], rhs=sk,
                        start=True, stop=False, skip_group_check=True,
                    )

            emit_qk(0)
            # ---------- rel_row chain: matmul -> ACT copies -> SBUF overlap DMA
            rr_ps = ps.tile([65, 511], f32, tag="s", bufs=4)
            nc.tensor.matmul(rr_ps, lhsT=w2b, rhs=rpT_b, start=True, stop=True)
            rr_sb = const.tile([1, 2, 511], bf16)
            for n in range(2):
                nc.scalar.copy(out=rr_sb[0:1, n, :], in_=rr_ps[n * 64:n * 64 + 1, :])
            nc.sync.dma_start(
                out=rrd[:].rearrange("(a b c) -> a b c", a=1, b=2), in_=rr_sb
            )
            # Toeplitz windows: overlapping DRAM reads (Fw[p,c] = rel_row[n,p+c])
            Fw = io.tile([128, 2, 384], bf16)
            for n in range(2):
                srcap = bass.AP(tensor=rrd, offset=n * 511, ap=[[1, 128], [1, 384]])
                nc.sync.dma_start(out=Fw[:, n, :], in_=srcap)

            cadd = {}
            for n in range(2):
                for ib in range(2):
                    off = 128 * (1 - ib)
                    ct = io.tile([128, 256], bf16, name=f"cadd_{n}_{ib}")
                    nc.vector.tensor_tensor(
                        ct, mneg_b[:, ib, :], Fw[:, n, off:off + 256],
                        mybir.AluOpType.add,
                    )
                    cadd[n, ib] = ct
            emit_qk(1)
            # wave A (global) finishers: single J matmul adds mask penalty
            for hl, ib in tiles:
                nc.tensor.matmul(
                    s_ps[0, hl, ib], lhsT=Jt, rhs=mneg_b[:, ib, :],
                    start=False, stop=True, skip_group_check=True,
                )
            for hp in range(2):
                local = hp == 1
                for hl in range(2):
                    p_sb[hp, hl] = work.tile(
                        [128, 2, 256], bf16, tag="p", bufs=3, name=f"p_{hp}_{hl}"
                    )
                    p_out[hp, hl] = work.tile(
                        [128, 2, 256], bf16, tag="po", bufs=2, name=f"po_{hp}_{hl}"
                    )
                    z[hp, hl] = work.tile(
                        [128, 2], f32, tag="z", bufs=2, name=f"z_{hp}_{hl}"
                    )
                    rz[hp, hl] = work.tile(
                        [128, 2], f32, tag="rz", bufs=2, name=f"rz_{hp}_{hl}"
                    )
                if local:
                    # wave B finishers: rank-1 a[j] + combined mask/Toeplitz J
                    for hl, ib in tiles:
                        nc.tensor.matmul(
                            s_ps[1, hl, ib], lhsT=onesb, rhs=a2_sb[hl],
                            start=False, stop=False, skip_group_check=True,
                        )
                    for hl, ib in tiles:
                        nc.tensor.matmul(
                            s_ps[1, hl, ib], lhsT=Jt, rhs=cadd[hl, ib],
                            start=False, stop=True, skip_group_check=True,
                        )
                for hl, ib in tiles:
                    nc.scalar.activation(
                        out=p_sb[hp, hl][:, ib, :], in_=s_ps[hp, hl, ib],
                        func=Exp, accum_out=z[hp, hl][:, ib:ib + 1],
                    )
                for hl in range(2):
                    nc.vector.reciprocal(out=rz[hp, hl], in_=z[hp, hl])

                # P^T via PE transposes of unnormalized p
                for hl in range(2):
                    for jb in range(2):
                        pt_ps = ps.tile(
                            [128, 256], bf16, tag="pt", bufs=2,
                            name=f"ptps_{hp}_{hl}_{jb}",
                        )
                        for ib in range(2):
                            nc.tensor.transpose(
                                pt_ps[:, ts(ib, 128)],
                                p_sb[hp, hl][:, ib, ts(jb, 128)], idb,
                            )
                        pt_sb = work.tile(
                            [128, 256], bf16, tag="ptsb", bufs=4,
                            name=f"ptsb_{hp}_{hl}_{jb}",
                        )
                        if jb == 0:
                            nc.vector.tensor_copy(out=pt_sb, in_=pt_ps)
                        else:
                            nc.scalar.copy(out=pt_sb, in_=pt_ps)
                        pT[hp, hl, jb] = pt_sb

                # O = P V, normalized by rowsums in the PSUM->SBUF copy
                for hl in range(2):
                    h = hp * 2 + hl
                    for ib in range(2):
                        o_ps = ps.tile(
                            [128, 64], f32, tag="o", bufs=2, name=f"o_{hp}_{hl}_{ib}"
                        )
                        for jb in range(2):
                            nc.tensor.matmul(
                                o_ps, lhsT=pT[hp, hl, jb][:, ts(ib, 128)],
                                rhs=v_b[:, h, jb, :],
                                start=(jb == 0), stop=(jb == 1),
                                skip_group_check=True,
                            )
                        nc.vector.tensor_scalar_mul(
                            o_all[:, h, ib, :], o_ps, rz[hp, hl][:, ib:ib + 1]
                        )

                # normalize p for output (separate tile: no WAR with pT reads)
                pdst = pg if not local else pl
                for hl in range(2):
                    for ib in range(2):
                        nc.vector.tensor_scalar_mul(
                            p_out[hp, hl][:, ib, :], p_sb[hp, hl][:, ib, :],
                            rz[hp, hl][:, ib:ib + 1],
                        )
                    nc.gpsimd.dma_start(out=pdst[hl], in_=p_out[hp, hl])

            nc.gpsimd.dma_start(out=outp[:], in_=o_all)

    nc.compile()
    return nc


def _shard_inputs(inputs):
    """Host-side layout transforms + batch sharding (no arithmetic)."""
    q = np.asarray(inputs["query"], dtype=np.float32)
    k = np.asarray(inputs["key"], dtype=np.float32)
    v = np.asarray(inputs["value"], dtype=np.float32)
    mask = np.asarray(inputs["mask"])
    apq = np.asarray(inputs["abs_pos_q"], dtype=np.float32).reshape(LN, L, DK)
    apk = np.asarray(inputs["abs_pos_k"], dtype=np.float32).reshape(LN, L, DK)
    relp = np.asarray(inputs["rel_pos"], dtype=np.float32)
    w = np.asarray(inputs["mlp_w"], dtype=np.float32)

    aqT = np.ascontiguousarray(apq.transpose(0, 2, 1).reshape(128, 256))
    akT = np.ascontiguousarray(apk.transpose(0, 2, 1).reshape(128, 256))
    rpT = np.ascontiguousarray(relp.T)
    w2h = np.zeros((128, 65), dtype=np.float32)
    for n in range(LN):
        w2h[n * 64:(n + 1) * 64, n * 64] = w[n]

    def dmaj(x):  # (4,256,64) -> (128=hl*64+d, hp, 256)
        t = x.transpose(0, 2, 1).reshape(2, 2, 64, 256).transpose(1, 2, 0, 3)
        return np.ascontiguousarray(t.reshape(128, 2, 256))

    in_maps = []
    for b in range(NCORES):
        m = {
            "qT": dmaj(q[b]),
            "kT": dmaj(k[b]),
            "vn": np.ascontiguousarray(
                v[b].reshape(4, 2, 128, 64).transpose(2, 0, 1, 3)
            ),
            "vTl": np.ascontiguousarray(
                v[b, GN:].transpose(0, 2, 1).reshape(128, 256)
            ),
            "msk": np.ascontiguousarray(
                mask[b, 0].reshape(2, 128, 256).transpose(1, 0, 2)[::-1]
            ).astype(np.uint8),
            "aqT": aqT,
            "akT": akT,
            "rpT": rpT,
            "w2": w2h,
        }
        in_maps.append(m)
    return in_maps


def _gather_outputs(results):
    outs, pgs, pls = [], [], []
    for res in results:
        o = np.asarray(res["out"], dtype=np.float32)
        outs.append(o.transpose(1, 2, 0, 3).reshape(4, 256, 64))
        pgs.append(
            np.asarray(res["p_g"]).astype(np.float32)
            .transpose(0, 2, 1, 3).reshape(2, 256, 256)
        )
        pls.append(
            np.asarray(res["p_l"]).astype(np.float32)
            .transpose(0, 2, 1, 3).reshape(2, 256, 256)
        )
    return np.stack(outs), np.stack(pgs), np.stack(pls)


def run_sharded(inputs, trace=False, **kw):
    from concourse.bass_utils import run_bass_kernel_spmd

    if "nc" not in _CACHE:
        _CACHE["nc"] = _build_nc()
    nc = _CACHE["nc"]
    in_maps = _shard_inputs(inputs)
    res = run_bass_kernel_spmd(
        nc, in_maps, core_ids=list(range(NCORES)), trace=trace, **kw
    )
    return _gather_outputs(res.results), res


def kernel(**inputs):
    (out, p_g, p_l), _ = run_sharded(inputs, trace=False)
    return out, p_g, p_l


# revision 29
# speedup vs baseline: 1.1340x; 1.1340x over previous
"""Trainium2 Bass kernel for sparse_attention (nn_Attention_77446850282057).

B=8, H=4, L=256, DK=64; 2 global heads + 2 local heads with per-head MLP
reweighting of pairwise (rel_pos + v_i + v_j) features.

Math: the MLP is linear, so
  reweight[b,n,i,j] = R[n,i,j] + a[b,n,i] + a[b,n,j] + mlp_b[n]
with R[n,i,j] = rel_pos[j-i+L-1, nDK:] . w[n]   (Toeplitz in (i,j))
and  a[b,n,j] = v_l[b,n,j] . w[n].
Row-constant terms (a[...,i] and mlp_b) shift softmax rows uniformly and
drop out exactly.  Softmax is computed without max-subtraction (scores are
bounded ~|S|<15 so raw exp is safe in f32); masked entries get -1e4 added
which underflows exp to exactly 0, matching the reference's where(mask,-1e4).

Sharding: pure data-parallel over batch, 1 batch per NeuronCore, 8 cores.

Device notes (per core):
 - host passes q/k/v/rel_pos transposed (d-major) — no input transposes
 - S tile (i-part, j-free) accumulated in PSUM by chained matmuls:
     qk (contract 64) + rank-1 a[j] (ones^T @ a2) + one J^T matmul adding
     mask penalty and Toeplitz together (host flips mask rows; J@J=I)
 - p = exp(S) via ACT with accum_out row sums; P^T via PE transposes of
   unnormalized p; O normalized by 1/rowsum in the PSUM->SBUF copy
 - DMA issue spread across sync (inputs), scalar (inputs), gpsimd
   (rel scratch round-trip + outputs) to avoid sequencer serialization
"""

import numpy as np

B, H, L, DK = 8, 4, 256, 64
GN, LN = 2, 2
SCALE = 1.0 / 8.0
NCORES = 8

_CACHE = {}


def _build_nc():
    import concourse.bass as bass
    import concourse.tile as tile
    from concourse import bacc, mybir
    from concourse.bass import ts
    from concourse.masks import make_identity

    f32 = mybir.dt.float32
    bf16 = mybir.dt.bfloat16
    u8 = mybir.dt.uint8

    # Bacc (not Bass): its compile pass splits multi-sem waits into
    # EventSemaphore instructions — TRN2 structs take at most one wait
    nc = bacc.Bacc()

    qT = nc.dram_tensor("qT", [128, 2, 256], f32, kind="ExternalInput")
    kT = nc.dram_tensor("kT", [128, 2, 256], f32, kind="ExternalInput")
    vn = nc.dram_tensor("vn", [128, 4, 2, 64], f32, kind="ExternalInput")
    vTl = nc.dram_tensor("vTl", [128, 256], f32, kind="ExternalInput")
    msk = nc.dram_tensor("msk", [128, 2, 256], u8, kind="ExternalInput")
    aqT = nc.dram_tensor("aqT", [128, 256], f32, kind="ExternalInput")
    akT = nc.dram_tensor("akT", [128, 256], f32, kind="ExternalInput")
    rpT = nc.dram_tensor("rpT", [128, 511], f32, kind="ExternalInput")
    # host-scattered weights: w2[n*64+d, n*64] = mlp_w[n, d], zeros elsewhere
    w2in = nc.dram_tensor("w2", [128, 65], f32, kind="ExternalInput")

    outp = nc.dram_tensor("out", [128, 4, 2, 64], f32, kind="ExternalOutput")
    rrd = nc.dram_tensor("rr_scratch", [2 * 511], bf16)
    pg = nc.dram_tensor("p_g", [2, 128, 2, 256], bf16, kind="ExternalOutput")
    pl = nc.dram_tensor("p_l", [2, 128, 2, 256], bf16, kind="ExternalOutput")

    Exp = mybir.ActivationFunctionType.Exp

    with tile.TileContext(nc) as tc:
        with (
            tc.tile_pool(name="const", bufs=1) as const,
            tc.tile_pool(name="io", bufs=1) as io,
            tc.tile_pool(name="work", bufs=2) as work,
            tc.tile_pool(name="ps", bufs=1, space="PSUM") as ps,
        ):
            # ---------- input loads: two HWDGE rings, small gating loads first
            mask_u = io.tile([128, 2, 256], u8)
            nc.sync.dma_start(out=mask_u, in_=msk[:])
            w2 = const.tile([128, 65], f32)
            nc.sync.dma_start(out=w2, in_=w2in[:])
            rpT_f = io.tile([128, 511], f32)
            nc.sync.dma_start(out=rpT_f, in_=rpT[:])
            v_f = io.tile([128, 4, 2, 64], f32)
            nc.sync.dma_start(out=v_f, in_=vn[:])

            qT_f = io.tile([128, 2, 256], f32)
            nc.scalar.dma_start(out=qT_f, in_=qT[:])
            kT_f = io.tile([128, 2, 256], f32)
            nc.scalar.dma_start(out=kT_f, in_=kT[:])
            aqT_f = io.tile([128, 256], f32)
            nc.scalar.dma_start(out=aqT_f, in_=aqT[:])
            akT_f = io.tile([128, 256], f32)
            nc.scalar.dma_start(out=akT_f, in_=akT[:])
            vTl_f = io.tile([128, 256], f32)
            nc.scalar.dma_start(out=vTl_f, in_=vTl[:])

            # ---------- constants
            idb = const.tile([128, 128], bf16)  # identity for PE transposes
            make_identity(nc, idb)
            Jt = const.tile([128, 128], bf16)  # anti-identity J[p,y]=1 iff p+y=127
            nc.gpsimd.memset(Jt, 0.0)
            nc.gpsimd.affine_select(
                out=Jt, in_=Jt, compare_op=mybir.AluOpType.not_equal,
                fill=1.0, base=-127, pattern=[[1, 128]], channel_multiplier=1,
            )
            onesb = const.tile([1, 128], bf16)
            nc.gpsimd.memset(onesb, 1.0)

            # ---------- DVE: q/k bf16 prep first (gates both waves' qk mms)
            qT_b = io.tile([128, 2, 256], bf16)
            kT_b = io.tile([128, 2, 256], bf16)
            nc.vector.tensor_scalar_mul(qT_b[:, 0, :], qT_f[:, 0, :], SCALE)
            nc.vector.tensor_copy(out=kT_b[:, 0, :], in_=kT_f[:, 0, :])
            qlt = io.tile([128, 256], f32)
            nc.vector.tensor_tensor(qlt, qT_f[:, 1, :], aqT_f, mybir.AluOpType.add)
            klt = io.tile([128, 256], f32)
            nc.vector.tensor_tensor(klt, kT_f[:, 1, :], akT_f, mybir.AluOpType.add)
            nc.vector.tensor_scalar_mul(qT_b[:, 1, :], qlt, SCALE)
            nc.vector.tensor_copy(out=kT_b[:, 1, :], in_=klt)

            # ---------- ACT: rel-chain casts + mask penalty (DVE queue is long)
            w2b = const.tile([128, 65], bf16)
            nc.scalar.copy(out=w2b, in_=w2)
            mneg_b = io.tile([128, 2, 256], bf16)
            nc.scalar.activation(
                out=mneg_b, in_=mask_u,
                func=mybir.ActivationFunctionType.Copy, scale=-10000.0,
            )
            rpT_b = io.tile([128, 511], bf16)
            nc.scalar.copy(out=rpT_b, in_=rpT_f)

            # ---------- attention: all S matmuls emitted in data-readiness
            # order (PE sequencers run in-order; a stalled op blocks the queue)
            o_all = const.tile([128, 4, 2, 64], f32)
            s_ps, p_sb, p_out, z, rz, pT = {}, {}, {}, {}, {}, {}
            tiles = [(hl, ib) for hl in range(2) for ib in range(2)]

            def emit_qk(hp):
                for hl, ib in tiles:
                    sp = ps.tile(
                        [128, 256], f32, tag="s", bufs=4, name=f"s_{hp}_{hl}_{ib}"
                    )
                    s_ps[hp, hl, ib] = sp
                    sq = qT_b[hl * 64:(hl + 1) * 64, hp, :]
                    sk = kT_b[hl * 64:(hl + 1) * 64, hp, :]
                    nc.tensor.matmul(
                        sp, lhsT=sq[:, ts(ib, 128)], rhs=sk,
                        start=True, stop=False, skip_group_check=True,
                    )

            emit_qk(0)
            # wave A (global) finishers: single J matmul adds mask penalty
            for hl, ib in tiles:
                nc.tensor.matmul(
                    s_ps[0, hl, ib], lhsT=Jt, rhs=mneg_b[:, ib, :],
                    start=False, stop=True, skip_group_check=True,
                )
            # ---------- rel_row chain: matmul -> ACT copies -> SBUF overlap DMA
            rr_ps = ps.tile([65, 511], f32, tag="s", bufs=4)
            nc.tensor.matmul(rr_ps, lhsT=w2b, rhs=rpT_b, start=True, stop=True)
            rr_sb = const.tile([1, 2, 511], bf16)
            for n in range(2):
                nc.scalar.copy(out=rr_sb[0:1, n, :], in_=rr_ps[n * 64:n * 64 + 1, :])
            nc.sync.dma_start(
                out=rrd[:].rearrange("(a b c) -> a b c", a=1, b=2), in_=rr_sb
            )
            # Toeplitz windows: overlapping DRAM reads (Fw[p,c] = rel_row[n,p+c])
            Fw = io.tile([128, 2, 384], bf16)
            for n in range(2):
                srcap = bass.AP(tensor=rrd, offset=n * 511, ap=[[1, 128], [1, 384]])
                nc.sync.dma_start(out=Fw[:, n, :], in_=srcap)

            # ---------- DVE: remaining converts + combined local J-rhs
            vT_b = io.tile([128, 256], bf16)
            nc.vector.tensor_copy(out=vT_b, in_=vTl_f)
            v_b = io.tile([128, 4, 2, 64], bf16)
            nc.vector.tensor_copy(out=v_b, in_=v_f)

            # ---------- a2 = w . v_l (both local heads, partitions 0/64)
            a2_ps = ps.tile([65, 256], f32, tag="s", bufs=4)
            nc.tensor.matmul(a2_ps, lhsT=w2b, rhs=vT_b, start=True, stop=True)
            a2_sb = [const.tile([1, 256], bf16, name=f"a2_{n}") for n in range(2)]
            for n in range(2):
                nc.vector.tensor_copy(out=a2_sb[n], in_=a2_ps[n * 64:n * 64 + 1, :])

            cadd = {}
            for n in range(2):
                for ib in range(2):
                    off = 128 * (1 - ib)
                    ct = io.tile([128, 256], bf16, name=f"cadd_{n}_{ib}")
                    nc.vector.tensor_tensor(
                        ct, mneg_b[:, ib, :], Fw[:, n, off:off + 256],
                        mybir.AluOpType.add,
                    )
                    cadd[n, ib] = ct
            emit_qk(1)
            for hp in range(2):
                local = hp == 1
                for hl in range(2):
                    p_sb[hp, hl] = work.tile(
                        [128, 2, 256], bf16, tag="p", bufs=3, name=f"p_{hp}_{hl}"
                    )
                    p_out[hp, hl] = work.tile(
                        [128, 2, 256], bf16, tag="po", bufs=2, name=f"po_{hp}_{hl}"
                    )
                    z[hp, hl] = work.tile(
                        [128, 2], f32, tag="z", bufs=2, name=f"z_{hp}_{hl}"
                    )
                    rz[hp, hl] = work.tile(
                        [128, 2], f32, tag="rz", bufs=2, name=f"rz_{hp}_{hl}"
                    )
                if local:
                    # wave B finishers: rank-1 a[j] + combined mask/Toeplitz J
                    for hl, ib in tiles:
                        nc.tensor.matmul(
                            s_ps[1, hl, ib], lhsT=onesb, rhs=a2_sb[hl],
                            start=False, stop=False, skip_group_check=True,
                        )
                    for hl, ib in tiles:
                        nc.tensor.matmul(
                            s_ps[1, hl, ib], lhsT=Jt, rhs=cadd[hl, ib],
                            start=False, stop=True, skip_group_check=True,
                        )
                for hl, ib in tiles:
                    nc.scalar.activation(
                        out=p_sb[hp, hl][:, ib, :], in_=s_ps[hp, hl, ib],
                        func=Exp, accum_out=z[hp, hl][:, ib:ib + 1],
                    )
                for hl in range(2):
                    nc.vector.reciprocal(out=rz[hp, hl], in_=z[hp, hl])

                # P^T via PE transposes of unnormalized p
                for hl in range(2):
                    for jb in range(2):
                        pt_ps = ps.tile(
                            [128, 256], bf16, tag="pt", bufs=2,
                            name=f"ptps_{hp}_{hl}_{jb}",
                        )
                        for ib in range(2):
                            nc.tensor.transpose(
                                pt_ps[:, ts(ib, 128)],
                                p_sb[hp, hl][:, ib, ts(jb, 128)], idb,
                            )
                        pt_sb = work.tile(
                            [128, 256], bf16, tag="ptsb", bufs=4,
                            name=f"ptsb_{hp}_{hl}_{jb}",
                        )
                        if jb == 0:
                            nc.vector.tensor_copy(out=pt_sb, in_=pt_ps)
                        else:
                            nc.scalar.copy(out=pt_sb, in_=pt_ps)
                        pT[hp, hl, jb] = pt_sb

                # O = P V, normalized by rowsums in the PSUM->SBUF copy
                for hl in range(2):
                    h = hp * 2 + hl
                    for ib in range(2):
                        o_ps = ps.tile(
                            [128, 64], f32, tag="o", bufs=2, name=f"o_{hp}_{hl}_{ib}"
                        )
                        for jb in range(2):
                            nc.tensor.matmul(
                                o_ps, lhsT=pT[hp, hl, jb][:, ts(ib, 128)],
                                rhs=v_b[:, h, jb, :],
                                start=(jb == 0), stop=(jb == 1),
                                skip_group_check=True,
                            )
                        nc.vector.tensor_scalar_mul(
                            o_all[:, h, ib, :], o_ps, rz[hp, hl][:, ib:ib + 1]
                        )

                # normalize p for output (separate tile: no WAR with pT reads)
                pdst = pg if not local else pl
                for hl in range(2):
                    for ib in range(2):
                        nc.vector.tensor_scalar_mul(
                            p_out[hp, hl][:, ib, :], p_sb[hp, hl][:, ib, :],
                            rz[hp, hl][:, ib:ib + 1],
                        )
                    nc.gpsimd.dma_start(out=pdst[hl], in_=p_out[hp, hl])

            nc.gpsimd.dma_start(out=outp[:], in_=o_all)

    nc.compile()
    return nc


def _shard_inputs(inputs):
    """Host-side layout transforms + batch sharding (no arithmetic)."""
    q = np.asarray(inputs["query"], dtype=np.float32)
    k = np.asarray(inputs["key"], dtype=np.float32)
    v = np.asarray(inputs["value"], dtype=np.float32)
    mask = np.asarray(inputs["mask"])
    apq = np.asarray(inputs["abs_pos_q"], dtype=np.float32).reshape(LN, L, DK)
    apk = np.asarray(inputs["abs_pos_k"], dtype=np.float32).reshape(LN, L, DK)
    relp = np.asarray(inputs["rel_pos"], dtype=np.float32)
    w = np.asarray(inputs["mlp_w"], dtype=np.float32)

    aqT = np.ascontiguousarray(apq.transpose(0, 2, 1).reshape(128, 256))
    akT = np.ascontiguousarray(apk.transpose(0, 2, 1).reshape(128, 256))
    rpT = np.ascontiguousarray(relp.T)
    w2h = np.zeros((128, 65), dtype=np.float32)
    for n in range(LN):
        w2h[n * 64:(n + 1) * 64, n * 64] = w[n]

    def dmaj(x):  # (4,256,64) -> (128=hl*64+d, hp, 256)
        t = x.transpose(0, 2, 1).reshape(2, 2, 64, 256).transpose(1, 2, 0, 3)
        return np.ascontiguousarray(t.reshape(128, 2, 256))

    in_maps = []
    for b in range(NCORES):
        m = {
            "qT": dmaj(q[b]),
            "kT": dmaj(k[b]),
            "vn": np.ascontiguousarray(
                v[b].reshape(4, 2, 128, 64).transpose(2, 0, 1, 3)
            ),
            "vTl": np.ascontiguousarray(
                v[b, GN:].transpose(0, 2, 1).reshape(128, 256)
            ),
            "msk": np.ascontiguousarray(
                mask[b, 0].reshape(2, 128, 256).transpose(1, 0, 2)[::-1]
            ).astype(np.uint8),
            "aqT": aqT,
            "akT": akT,
            "rpT": rpT,
            "w2": w2h,
        }
        in_maps.append(m)
    return in_maps


def _gather_outputs(results):
    outs, pgs, pls = [], [], []
    for res in results:
        o = np.asarray(res["out"], dtype=np.float32)
        outs.append(o.transpose(1, 2, 0, 3).reshape(4, 256, 64))
        pgs.append(
            np.asarray(res["p_g"]).astype(np.float32)
            .transpose(0, 2, 1, 3).reshape(2, 256, 256)
        )
        pls.append(
            np.asarray(res["p_l"]).astype(np.float32)
            .transpose(0, 2, 1, 3).reshape(2, 256, 256)
        )
    return np.stack(outs), np.stack(pgs), np.stack(pls)


def run_sharded(inputs, trace=False, **kw):
    from concourse.bass_utils import run_bass_kernel_spmd

    if "nc" not in _CACHE:
        _CACHE["nc"] = _build_nc()
    nc = _CACHE["nc"]
    in_maps = _shard_inputs(inputs)
    res = run_bass_kernel_spmd(
        nc, in_maps, core_ids=list(range(NCORES)), trace=trace, **kw
    )
    return _gather_outputs(res.results), res


def kernel(**inputs):
    (out, p_g, p_l), _ = run_sharded(inputs, trace=False)
    return out, p_g, p_l


# revision 30
# speedup vs baseline: 1.1500x; 1.0140x over previous
"""Trainium2 Bass kernel for sparse_attention (nn_Attention_77446850282057).

B=8, H=4, L=256, DK=64; 2 global heads + 2 local heads with per-head MLP
reweighting of pairwise (rel_pos + v_i + v_j) features.

Math: the MLP is linear, so
  reweight[b,n,i,j] = R[n,i,j] + a[b,n,i] + a[b,n,j] + mlp_b[n]
with R[n,i,j] = rel_pos[j-i+L-1, nDK:] . w[n]   (Toeplitz in (i,j))
and  a[b,n,j] = v_l[b,n,j] . w[n].
Row-constant terms (a[...,i] and mlp_b) shift softmax rows uniformly and
drop out exactly.  Softmax is computed without max-subtraction (scores are
bounded ~|S|<15 so raw exp is safe in f32); masked entries get -1e4 added
which underflows exp to exactly 0, matching the reference's where(mask,-1e4).

Sharding: pure data-parallel over batch, 1 batch per NeuronCore, 8 cores.

Device notes (per core):
 - host passes q/k/v/rel_pos transposed (d-major) — no input transposes
 - S tile (i-part, j-free) accumulated in PSUM by chained matmuls:
     qk (contract 64) + rank-1 a[j] (ones^T @ a2) + one J^T matmul adding
     mask penalty and Toeplitz together (host flips mask rows; J@J=I)
 - p = exp(S) via ACT with accum_out row sums; P^T via PE transposes of
   unnormalized p; O normalized by 1/rowsum in the PSUM->SBUF copy
 - DMA issue spread across sync (inputs), scalar (inputs), gpsimd
   (rel scratch round-trip + outputs) to avoid sequencer serialization
"""

import numpy as np

B, H, L, DK = 8, 4, 256, 64
GN, LN = 2, 2
SCALE = 1.0 / 8.0
NCORES = 8

_CACHE = {}


def _build_nc():
    import concourse.bass as bass
    import concourse.tile as tile
    from concourse import bacc, mybir
    from concourse.bass import ts
    from concourse.masks import make_identity

    f32 = mybir.dt.float32
    bf16 = mybir.dt.bfloat16
    u8 = mybir.dt.uint8

    # Bacc (not Bass): its compile pass splits multi-sem waits into
    # EventSemaphore instructions — TRN2 structs take at most one wait
    nc = bacc.Bacc()

    qT = nc.dram_tensor("qT", [128, 2, 256], f32, kind="ExternalInput")
    kT = nc.dram_tensor("kT", [128, 2, 256], f32, kind="ExternalInput")
    vn = nc.dram_tensor("vn", [128, 4, 2, 64], f32, kind="ExternalInput")
    vTl = nc.dram_tensor("vTl", [128, 256], f32, kind="ExternalInput")
    msk = nc.dram_tensor("msk", [128, 2, 256], u8, kind="ExternalInput")
    aqT = nc.dram_tensor("aqT", [128, 256], f32, kind="ExternalInput")
    akT = nc.dram_tensor("akT", [128, 256], f32, kind="ExternalInput")
    rpT = nc.dram_tensor("rpT", [128, 511], f32, kind="ExternalInput")
    # host-scattered weights: w2[n*64+d, n*64] = mlp_w[n, d], zeros elsewhere
    w2in = nc.dram_tensor("w2", [128, 65], f32, kind="ExternalInput")

    outp = nc.dram_tensor("out", [128, 4, 2, 64], f32, kind="ExternalOutput")
    rrd = nc.dram_tensor("rr_scratch", [2 * 511], bf16)
    pg = nc.dram_tensor("p_g", [2, 128, 2, 256], bf16, kind="ExternalOutput")
    pl = nc.dram_tensor("p_l", [2, 128, 2, 256], bf16, kind="ExternalOutput")

    Exp = mybir.ActivationFunctionType.Exp

    with tile.TileContext(nc) as tc:
        with (
            tc.tile_pool(name="const", bufs=1) as const,
            tc.tile_pool(name="io", bufs=1) as io,
            tc.tile_pool(name="work", bufs=2) as work,
            tc.tile_pool(name="ps", bufs=1, space="PSUM") as ps,
        ):
            # ---------- input loads: two HWDGE rings; rel-chain + gating first
            rpT_f = io.tile([128, 511], f32)
            nc.sync.dma_start(out=rpT_f, in_=rpT[:])
            w2 = const.tile([128, 65], f32)
            nc.sync.dma_start(out=w2, in_=w2in[:])
            mask_u = io.tile([128, 2, 256], u8)
            nc.sync.dma_start(out=mask_u, in_=msk[:])
            vTl_f = io.tile([128, 256], f32)
            nc.sync.dma_start(out=vTl_f, in_=vTl[:])
            v_f = io.tile([128, 4, 2, 64], f32)
            nc.sync.dma_start(out=v_f, in_=vn[:])

            qT_f = io.tile([128, 2, 256], f32)
            nc.scalar.dma_start(out=qT_f, in_=qT[:])
            kT_f = io.tile([128, 2, 256], f32)
            nc.scalar.dma_start(out=kT_f, in_=kT[:])
            aqT_f = io.tile([128, 256], f32)
            nc.scalar.dma_start(out=aqT_f, in_=aqT[:])
            akT_f = io.tile([128, 256], f32)
            nc.scalar.dma_start(out=akT_f, in_=akT[:])

            # ---------- constants
            idb = const.tile([128, 128], bf16)  # identity for PE transposes
            make_identity(nc, idb)
            Jt = const.tile([128, 128], bf16)  # anti-identity J[p,y]=1 iff p+y=127
            nc.gpsimd.memset(Jt, 0.0)
            nc.gpsimd.affine_select(
                out=Jt, in_=Jt, compare_op=mybir.AluOpType.not_equal,
                fill=1.0, base=-127, pattern=[[1, 128]], channel_multiplier=1,
            )
            onesb = const.tile([1, 128], bf16)
            nc.gpsimd.memset(onesb, 1.0)

            # ---------- DVE: q/k bf16 prep first (gates both waves' qk mms)
            qT_b = io.tile([128, 2, 256], bf16)
            kT_b = io.tile([128, 2, 256], bf16)
            nc.vector.tensor_scalar_mul(qT_b[:, 0, :], qT_f[:, 0, :], SCALE)
            nc.vector.tensor_copy(out=kT_b[:, 0, :], in_=kT_f[:, 0, :])
            qlt = io.tile([128, 256], f32)
            nc.vector.tensor_tensor(qlt, qT_f[:, 1, :], aqT_f, mybir.AluOpType.add)
            klt = io.tile([128, 256], f32)
            nc.vector.tensor_tensor(klt, kT_f[:, 1, :], akT_f, mybir.AluOpType.add)
            nc.vector.tensor_scalar_mul(qT_b[:, 1, :], qlt, SCALE)
            nc.vector.tensor_copy(out=kT_b[:, 1, :], in_=klt)

            # ---------- ACT: rel-chain casts + mask penalty (DVE queue is long)
            w2b = const.tile([128, 65], bf16)
            nc.scalar.copy(out=w2b, in_=w2)
            rpT_b = io.tile([128, 511], bf16)
            nc.scalar.copy(out=rpT_b, in_=rpT_f)
            mneg_b = io.tile([128, 2, 256], bf16)
            nc.scalar.activation(
                out=mneg_b, in_=mask_u,
                func=mybir.ActivationFunctionType.Copy, scale=-10000.0,
            )

            # ---------- attention: all S matmuls emitted in data-readiness
            # order (PE sequencers run in-order; a stalled op blocks the queue)
            o_all = const.tile([128, 4, 2, 64], f32)
            s_ps, p_sb, p_out, z, rz, pT = {}, {}, {}, {}, {}, {}
            tiles = [(hl, ib) for hl in range(2) for ib in range(2)]

            def emit_qk(hp):
                for hl, ib in tiles:
                    sp = ps.tile(
                        [128, 256], f32, tag="s", bufs=4, name=f"s_{hp}_{hl}_{ib}"
                    )
                    s_ps[hp, hl, ib] = sp
                    sq = qT_b[hl * 64:(hl + 1) * 64, hp, :]
                    sk = kT_b[hl * 64:(hl + 1) * 64, hp, :]
                    nc.tensor.matmul(
                        sp, lhsT=sq[:, ts(ib, 128)], rhs=sk,
                        start=True, stop=False, skip_group_check=True,
                    )

            emit_qk(0)
            # wave A (global) finishers: single J matmul adds mask penalty
            for hl, ib in tiles:
                nc.tensor.matmul(
                    s_ps[0, hl, ib], lhsT=Jt, rhs=mneg_b[:, ib, :],
                    start=False, stop=True, skip_group_check=True,
                )
            # ---------- rel_row chain: matmul -> ACT copies -> SBUF overlap DMA
            rr_ps = ps.tile([65, 511], f32, tag="s", bufs=4)
            nc.tensor.matmul(rr_ps, lhsT=w2b, rhs=rpT_b, start=True, stop=True)
            rr_sb = const.tile([1, 2, 511], bf16)
            for n in range(2):
                nc.scalar.copy(out=rr_sb[0:1, n, :], in_=rr_ps[n * 64:n * 64 + 1, :])
            nc.sync.dma_start(
                out=rrd[:].rearrange("(a b c) -> a b c", a=1, b=2), in_=rr_sb
            )
            # Toeplitz windows: overlapping DRAM reads (Fw[p,c] = rel_row[n,p+c])
            Fw = io.tile([128, 2, 384], bf16)
            for n in range(2):
                srcap = bass.AP(tensor=rrd, offset=n * 511, ap=[[1, 128], [1, 384]])
                nc.sync.dma_start(out=Fw[:, n, :], in_=srcap)

            # ---------- DVE: remaining converts + combined local J-rhs
            vT_b = io.tile([128, 256], bf16)
            nc.vector.tensor_copy(out=vT_b, in_=vTl_f)
            v_b = io.tile([128, 4, 2, 64], bf16)
            nc.vector.tensor_copy(out=v_b, in_=v_f)

            # ---------- a2 = w . v_l (both local heads, partitions 0/64)
            a2_ps = ps.tile([65, 256], f32, tag="s", bufs=4)
            nc.tensor.matmul(a2_ps, lhsT=w2b, rhs=vT_b, start=True, stop=True)
            a2_sb = [const.tile([1, 256], bf16, name=f"a2_{n}") for n in range(2)]
            for n in range(2):
                nc.vector.tensor_copy(out=a2_sb[n], in_=a2_ps[n * 64:n * 64 + 1, :])

            cadd = {}
            for n in range(2):
                for ib in range(2):
                    off = 128 * (1 - ib)
                    ct = io.tile([128, 256], bf16, name=f"cadd_{n}_{ib}")
                    nc.vector.tensor_tensor(
                        ct, mneg_b[:, ib, :], Fw[:, n, off:off + 256],
                        mybir.AluOpType.add,
                    )
                    cadd[n, ib] = ct
            emit_qk(1)
            for hp in range(2):
                local = hp == 1
                for hl in range(2):
                    p_sb[hp, hl] = work.tile(
                        [128, 2, 256], bf16, tag="p", bufs=3, name=f"p_{hp}_{hl}"
                    )
                    p_out[hp, hl] = work.tile(
                        [128, 2, 256], bf16, tag="po", bufs=2, name=f"po_{hp}_{hl}"
                    )
                    z[hp, hl] = work.tile(
                        [128, 2], f32, tag="z", bufs=2, name=f"z_{hp}_{hl}"
                    )
                    rz[hp, hl] = work.tile(
                        [128, 2], f32, tag="rz", bufs=2, name=f"rz_{hp}_{hl}"
                    )
                if local:
                    # wave B finishers: rank-1 a[j] + combined mask/Toeplitz J
                    for hl, ib in tiles:
                        nc.tensor.matmul(
                            s_ps[1, hl, ib], lhsT=onesb, rhs=a2_sb[hl],
                            start=False, stop=False, skip_group_check=True,
                        )
                    for hl, ib in tiles:
                        nc.tensor.matmul(
                            s_ps[1, hl, ib], lhsT=Jt, rhs=cadd[hl, ib],
                            start=False, stop=True, skip_group_check=True,
                        )
                for hl, ib in tiles:
                    nc.scalar.activation(
                        out=p_sb[hp, hl][:, ib, :], in_=s_ps[hp, hl, ib],
                        func=Exp, accum_out=z[hp, hl][:, ib:ib + 1],
                    )
                for hl in range(2):
                    nc.vector.reciprocal(out=rz[hp, hl], in_=z[hp, hl])

                # P^T via PE transposes of unnormalized p
                for hl in range(2):
                    for jb in range(2):
                        pt_ps = ps.tile(
                            [128, 256], bf16, tag="pt", bufs=2,
                            name=f"ptps_{hp}_{hl}_{jb}",
                        )
                        for ib in range(2):
                            nc.tensor.transpose(
                                pt_ps[:, ts(ib, 128)],
                                p_sb[hp, hl][:, ib, ts(jb, 128)], idb,
                            )
                        pt_sb = work.tile(
                            [128, 256], bf16, tag="ptsb", bufs=4,
                            name=f"ptsb_{hp}_{hl}_{jb}",
                        )
                        if jb == 0:
                            nc.vector.tensor_copy(out=pt_sb, in_=pt_ps)
                        else:
                            nc.scalar.copy(out=pt_sb, in_=pt_ps)
                        pT[hp, hl, jb] = pt_sb

                # O = P V, normalized by rowsums in the PSUM->SBUF copy
                for hl in range(2):
                    h = hp * 2 + hl
                    for ib in range(2):
                        o_ps = ps.tile(
                            [128, 64], f32, tag="o", bufs=2, name=f"o_{hp}_{hl}_{ib}"
                        )
                        for jb in range(2):
                            nc.tensor.matmul(
                                o_ps, lhsT=pT[hp, hl, jb][:, ts(ib, 128)],
                                rhs=v_b[:, h, jb, :],
                                start=(jb == 0), stop=(jb == 1),
                                skip_group_check=True,
                            )
                        nc.vector.tensor_scalar_mul(
                            o_all[:, h, ib, :], o_ps, rz[hp, hl][:, ib:ib + 1]
                        )

                # normalize p for output (separate tile: no WAR with pT reads)
                pdst = pg if not local else pl
                for hl in range(2):
                    for ib in range(2):
                        nc.vector.tensor_scalar_mul(
                            p_out[hp, hl][:, ib, :], p_sb[hp, hl][:, ib, :],
                            rz[hp, hl][:, ib:ib + 1],
                        )
                    nc.gpsimd.dma_start(out=pdst[hl], in_=p_out[hp, hl])

            nc.gpsimd.dma_start(out=outp[:], in_=o_all)

    nc.compile()
    return nc


def _shard_inputs(inputs):
    """Host-side layout transforms + batch sharding (no arithmetic)."""
    q = np.asarray(inputs["query"], dtype=np.float32)
    k = np.asarray(inputs["key"], dtype=np.float32)
    v = np.asarray(inputs["value"], dtype=np.float32)
    mask = np.asarray(inputs["mask"])
    apq = np.asarray(inputs["abs_pos_q"], dtype=np.float32).reshape(LN, L, DK)
    apk = np.asarray(inputs["abs_pos_k"], dtype=np.float32).reshape(LN, L, DK)
    relp = np.asarray(inputs["rel_pos"], dtype=np.float32)
    w = np.asarray(inputs["mlp_w"], dtype=np.float32)

    aqT = np.ascontiguousarray(apq.transpose(0, 2, 1).reshape(128, 256))
    akT = np.ascontiguousarray(apk.transpose(0, 2, 1).reshape(128, 256))
    rpT = np.ascontiguousarray(relp.T)
    w2h = np.zeros((128, 65), dtype=np.float32)
    for n in range(LN):
        w2h[n * 64:(n + 1) * 64, n * 64] = w[n]

    def dmaj(x):  # (4,256,64) -> (128=hl*64+d, hp, 256)
        t = x.transpose(0, 2, 1).reshape(2, 2, 64, 256).transpose(1, 2, 0, 3)
        return np.ascontiguousarray(t.reshape(128, 2, 256))

    in_maps = []
    for b in range(NCORES):
        m = {
            "qT": dmaj(q[b]),
            "kT": dmaj(k[b]),
            "vn": np.ascontiguousarray(
                v[b].reshape(4, 2, 128, 64).transpose(2, 0, 1, 3)
            ),
            "vTl": np.ascontiguousarray(
                v[b, GN:].transpose(0, 2, 1).reshape(128, 256)
            ),
            "msk": np.ascontiguousarray(
                mask[b, 0].reshape(2, 128, 256).transpose(1, 0, 2)[::-1]
            ).astype(np.uint8),
            "aqT": aqT,
            "akT": akT,
            "rpT": rpT,
            "w2": w2h,
        }
        in_maps.append(m)
    return in_maps


def _gather_outputs(results):
    outs, pgs, pls = [], [], []
    for res in results:
        o = np.asarray(res["out"], dtype=np.float32)
        outs.append(o.transpose(1, 2, 0, 3).reshape(4, 256, 64))
        pgs.append(
            np.asarray(res["p_g"]).astype(np.float32)
            .transpose(0, 2, 1, 3).reshape(2, 256, 256)
        )
        pls.append(
            np.asarray(res["p_l"]).astype(np.float32)
            .transpose(0, 2, 1, 3).reshape(2, 256, 256)
        )
    return np.stack(outs), np.stack(pgs), np.stack(pls)


def run_sharded(inputs, trace=False, **kw):
    from concourse.bass_utils import run_bass_kernel_spmd

    if "nc" not in _CACHE:
        _CACHE["nc"] = _build_nc()
    nc = _CACHE["nc"]
    in_maps = _shard_inputs(inputs)
    res = run_bass_kernel_spmd(
        nc, in_maps, core_ids=list(range(NCORES)), trace=trace, **kw
    )
    return _gather_outputs(res.results), res


def kernel(**inputs):
    (out, p_g, p_l), _ = run_sharded(inputs, trace=False)
    return out, p_g, p_l
